# revision 12
# baseline (speedup 1.0000x reference)
# Differential multi-head attention (dual softmax + GroupNorm + sigmoid gating)
# for Trainium2, batch-parallel across 8 NeuronCores (one batch row per core).
#
# Per-core math (batch b):
#   q = query @ Wq + bq -> per head: q1, q2, gate (each S x 64)
#   k = key   @ Wk + bk -> per head: k1, k2
#   v = values@ Wv + bv -> per head: v (S x 64)
#   attn = softmax(q1 k1^T / 8) - lam * softmax(q2 k2^T / 8)
#   out  = GroupNorm_{8 groups over d, reduced over (S, heads, d-in-group)}(attn @ v)
#   out  = out * (1 - lambda_init) * sigmoid(gate)
#
# Layout strategy:
#  - Host pre-packs x^T and all weights as bf16 (layout marshalling only; the
#    math runs on device).  q/k projections are d-major (q1/q2 resp. k1/k2 in
#    complementary 64-partition halves of one [128,S] tile); score matmuls are
#    K=64 at PE row-tiles 0/64 (no zero padding needed).
#  - "Flipped" o-matmul: stationary = exp-score slice [128k x 128q], moving =
#    v_aug [128k x 65] -> out is q-major [128q, 65] with the exp row-sum in
#    column 64 (ones column in v_aug).  This halves PE time vs the d-major
#    o-matmul, makes softmax normalization a per-partition scalar multiply,
#    and leaves y in the exact output layout (no final transposes).
#  - GroupNorm stats via ones-column f32r matmuls (Sum y, Sum y^2 over seq on
#    partitions), group-reduced on DVE, rsqrt via sqrt+NR, then the per-column
#    affine A,B is expanded back to [128,512] with tiny K=1/K=8 matmuls.
#  - ACT runs (nearly) only the 128 [128,1024] exp instructions + 8 tanh; all
#    projection epilogues run on DVE/Pool.  PSUM: s double-buffered (4 banks),
#    o manually packed into 3 banks (7+7+2 groups of 65 cols), proj 1 bank.

import numpy as np

B, S, H, D = 8, 1024, 8, 64
DM = H * D  # 512
NJ = S // 128   # key 128-tiles
NQ = S // 128   # query 128-blocks
EPS = 1e-3
INV = 0.125
CNT = float(S * H)  # groupnorm count per group = S * H * (D//H) / ... = 1024*8


def build_nc():
    import concourse.bacc as bacc
    import concourse.tile as tile
    from concourse import mybir

    f32 = mybir.dt.float32
    f32r = mybir.dt.float32r
    bf16 = mybir.dt.bfloat16
    AF = mybir.ActivationFunctionType
    OP = mybir.AluOpType
    AX = mybir.AxisListType

    nc = bacc.Bacc(target_bir_lowering=False)
    xq_d = nc.dram_tensor("xq", [128, 4 * S], bf16, kind="ExternalInput")
    xk_d = nc.dram_tensor("xk", [128, 4 * S], bf16, kind="ExternalInput")
    xv_d = nc.dram_tensor("xv", [128, 4 * S], bf16, kind="ExternalInput")
    wq_d = nc.dram_tensor("wq", [128, 4 * 1536], bf16, kind="ExternalInput")
    wk_d = nc.dram_tensor("wk", [128, 4 * 1024], bf16, kind="ExternalInput")
    wv_d = nc.dram_tensor("wv", [128, 4 * 512], bf16, kind="ExternalInput")
    wg_d = nc.dram_tensor("wg", [128, 4 * 512], bf16, kind="ExternalInput")
    bqk_d = nc.dram_tensor("bqk", [128, 16], f32, kind="ExternalInput")
    rows_d = nc.dram_tensor("rows", [1, 2048], f32, kind="ExternalInput")
    rowsb_d = nc.dram_tensor("rowsb", [1, 1024], bf16, kind="ExternalInput")
    g8_d = nc.dram_tensor("g8", [8, 512], f32, kind="ExternalInput")
    negl_d = nc.dram_tensor("negl", [1], f32, kind="ExternalInput")
    out_d = nc.dram_tensor("out", [S, DM], f32, kind="ExternalOutput")

    ts_ = nc.vector.tensor_scalar
    stt = nc.vector.scalar_tensor_tensor
    gts_ = nc.gpsimd.tensor_scalar
    gstt = nc.gpsimd.scalar_tensor_tensor

    # (t, qb) accumulation group -> (o-bank index, column offset); 65 cols per
    # group (64 d + 1 ones-sum), packed 7 + 7 + 2 into three psum banks.
    def o_loc(t, qb):
        g = t * 8 + qb
        if g < 7:
            return 0, 65 * g
        if g < 14:
            return 1, 65 * (g - 7)
        return 2, 65 * (g - 14)

    with tile.TileContext(nc) as tc:
        with tc.tile_pool(name="persist", bufs=1) as pp:
            # ---------- persistent SBUF ----------
            xq_t = pp.tile([128, 4 * S], bf16, tag="xq_t", name="xq_t")
            xk_t = pp.tile([128, 4 * S], bf16, tag="xk_t", name="xk_t")
            xv_t = pp.tile([128, 4 * S], bf16, tag="xv_t", name="xv_t")
            wq_t = pp.tile([128, 4 * 1536], bf16, tag="wq_t", name="wq_t")
            wk_t = pp.tile([128, 4 * 1024], bf16, tag="wk_t", name="wk_t")
            wv_t = pp.tile([128, 4 * 512], bf16, tag="wv_t", name="wv_t")
            wg_t = pp.tile([128, 4 * 512], bf16, tag="wg_t", name="wg_t")
            bqk_t = pp.tile([128, 16], f32, tag="bqk_t", name="bqk_t")
            rows_t = pp.tile([1, 2048], f32, tag="rows_t", name="rows_t")
            rowsb_t = pp.tile([1, 1024], bf16, tag="rowsb_t", name="rowsb_t")
            g8_t = pp.tile([8, 512], f32, tag="g8_t", name="g8_t")
            negl_t = pp.tile([128, 1], f32, tag="negl_t", name="negl_t")
            ones_c = pp.tile([128, 1], bf16, tag="ones_c", name="ones_c")
            ones_r = pp.tile([1, 128], bf16, tag="ones_r", name="ones_r")
            ones_rf = pp.tile([1, 128], f32, tag="ones_rf", name="ones_rf")
            one1 = pp.tile([1, 1], f32, tag="one1", name="one1")
            qp = [pp.tile([128, S], bf16, tag=f"qp{h}", name=f"qp{h}") for h in range(8)]
            kp = [pp.tile([128, S], bf16, tag=f"kp{h}", name=f"kp{h}") for h in range(8)]
            va = [pp.tile([128, 8, 65], bf16, tag=f"va{j}", name=f"va{j}") for j in range(NJ)]
            th = [pp.tile([128, 512], f32, tag=f"th{q}", name=f"th{q}") for q in range(NQ)]
            yy = [pp.tile([128, 512], f32, tag=f"yy{q}", name=f"yy{q}") for q in range(NQ)]
            mrst = pp.tile([8, 2], f32, tag="mrst", name="mrst")

            # ---------- DMAs ----------
            # SP queue: x tensors, n-half-first so the first score matmuls
            # can start as soon as ~1MB has landed.  ACT queue: weights,
            # heads-0/1 slices of wq/wk prefetched ahead of the bulk.
            xqv = xq_t.rearrange("p (r s) -> p r s", s=S)
            xkv = xk_t.rearrange("p (r s) -> p r s", s=S)
            xqd = xq_d[:, :].rearrange("p (r s) -> p r s", s=S)
            xkd = xk_d[:, :].rearrange("p (r s) -> p r s", s=S)
            nc.sync.dma_start(out=xqv[:, :, 0:512], in_=xqd[:, :, 0:512])
            nc.sync.dma_start(out=xkv[:, :, 0:512], in_=xkd[:, :, 0:512])
            nc.sync.dma_start(out=xqv[:, :, 512:1024], in_=xqd[:, :, 512:1024])
            nc.sync.dma_start(out=xkv[:, :, 512:1024], in_=xkd[:, :, 512:1024])
            nc.sync.dma_start(out=xv_t, in_=xv_d[:, :])
            wqv = wq_t.rearrange("p (r c) -> p r c", c=1536)
            wkv = wk_t.rearrange("p (r c) -> p r c", c=1024)
            wqd = wq_d[:, :].rearrange("p (r c) -> p r c", c=1536)
            wkd = wk_d[:, :].rearrange("p (r c) -> p r c", c=1024)
            nc.scalar.dma_start(out=wqv[:, :, 0:384], in_=wqd[:, :, 0:384])
            nc.scalar.dma_start(out=wkv[:, :, 0:256], in_=wkd[:, :, 0:256])
            nc.scalar.dma_start(out=wqv[:, :, 384:1536], in_=wqd[:, :, 384:1536])
            nc.scalar.dma_start(out=wkv[:, :, 256:1024], in_=wkd[:, :, 256:1024])
            nc.scalar.dma_start(out=wv_t, in_=wv_d[:, :])
            nc.scalar.dma_start(out=wg_t, in_=wg_d[:, :])
            nc.gpsimd.dma_start(out=bqk_t, in_=bqk_d[:, :])
            nc.gpsimd.dma_start(out=rows_t, in_=rows_d[:, :])
            nc.gpsimd.dma_start(out=rowsb_t, in_=rowsb_d[:, :])
            nc.gpsimd.dma_start(out=g8_t, in_=g8_d[:, :])
            nc.gpsimd.dma_start(out=negl_t, in_=negl_d[:].to_broadcast([128, 1]))
            nc.gpsimd.memset(ones_c, 1.0)
            nc.gpsimd.memset(ones_r, 1.0)
            nc.gpsimd.memset(ones_rf, 1.0)
            nc.gpsimd.memset(one1, 1.0)
            for j in range(NJ):
                nc.gpsimd.memset(va[j][:, :, 64:65], 1.0)

            ghl_r = rows_t[:, 0:512]
            bhl_r = rows_t[:, 512:1024]
            gb_r = rowsb_t[:, 0:512]
            vb_r = rowsb_t[:, 512:1024]

            with tc.tile_pool(name="ps_proj", bufs=1, space="PSUM") as ps_proj, \
                 tc.tile_pool(name="ps_s", bufs=2, space="PSUM") as ps_s, \
                 tc.tile_pool(name="ps_o", bufs=1, space="PSUM") as ps_o, \
                 tc.tile_pool(name="expp", bufs=3) as expp, \
                 tc.tile_pool(name="rscp", bufs=2) as rscp:

                # ---------- projection emitters ----------
                # qk projections for heads >= 2, v, and gate are emitted as
                # small "filler units" (1-2 PE matmuls each) interleaved into
                # the attention loop so the PE stream stays dense while the
                # loop is ACT(exp)-paced.
                def qk_units(h, kind):
                    if kind == "q":
                        w_t, x_t, dst = wq_t, xq_t, qp[h]
                        bcol = bqk_t[:, h:h + 1]
                        wofs = lambda r: 1536 * r + 192 * h
                    else:
                        w_t, x_t, dst = wk_t, xk_t, kp[h]
                        bcol = bqk_t[:, 8 + h:9 + h]
                        wofs = lambda r: 1024 * r + 128 * h
                    st = {}

                    def unit_a(n):
                        def f():
                            st[n] = ps_proj.tile([128, 512], f32, tag="proj",
                                                 name="proj")
                            for r in (0, 1):
                                nc.tensor.matmul(
                                    st[n], w_t[:, wofs(r):wofs(r) + 128],
                                    x_t[:, S * r + 512 * n:S * r + 512 * (n + 1)],
                                    start=(r == 0), stop=False)
                        return f

                    def unit_b(n):
                        def f():
                            for r in (2, 3):
                                nc.tensor.matmul(
                                    st[n], w_t[:, wofs(r):wofs(r) + 128],
                                    x_t[:, S * r + 512 * n:S * r + 512 * (n + 1)],
                                    start=False, stop=(r == 3))
                            ts_(dst[:, 512 * n:512 * (n + 1)], st[n],
                                bcol, None, OP.add)
                        return f
                    return [unit_a(0), unit_b(0), unit_a(1), unit_b(1)]

                def v_units(j):
                    st = {}

                    def unit_a():
                        st[0] = ps_proj.tile([128, 512], f32, tag="proj",
                                             name="proj")
                        for r in (0, 1):
                            nc.tensor.matmul(
                                st[0], xv_t[:, S * r + 128 * j:S * r + 128 * (j + 1)],
                                wv_t[:, 512 * r:512 * (r + 1)],
                                start=(r == 0), stop=False)

                    def unit_b():
                        for r in (2, 3):
                            nc.tensor.matmul(
                                st[0], xv_t[:, S * r + 128 * j:S * r + 128 * (j + 1)],
                                wv_t[:, 512 * r:512 * (r + 1)],
                                start=False, stop=False)
                        nc.tensor.matmul(st[0], ones_r[:], vb_r,
                                         start=False, stop=True)
                        nc.vector.tensor_copy(
                            va[j][:, :, 0:64],
                            st[0].rearrange("p (h d) -> p h d", d=64))
                    return [unit_a, unit_b]

                def g_units(q):
                    st = {}

                    def unit_a():
                        st[0] = ps_proj.tile([128, 512], f32, tag="proj",
                                             name="proj")
                        for r in (0, 1):
                            nc.tensor.matmul(
                                st[0], xq_t[:, S * r + 128 * q:S * r + 128 * (q + 1)],
                                wg_t[:, 512 * r:512 * (r + 1)],
                                start=(r == 0), stop=False)

                    def unit_b():
                        for r in (2, 3):
                            nc.tensor.matmul(
                                st[0], xq_t[:, S * r + 128 * q:S * r + 128 * (q + 1)],
                                wg_t[:, 512 * r:512 * (r + 1)],
                                start=False, stop=False)
                        nc.tensor.matmul(st[0], ones_r[:], gb_r,
                                         start=False, stop=True)
                        nc.scalar.activation(th[q], st[0], AF.Tanh, scale=0.5)
                    return [unit_a, unit_b]

                # ---------- combine: y[:, 64h:64h+64] = o1*r1 - lam*o2*r2 --
                def combine(h, ot):
                    rsc = rscp.tile([128, 16], f32, tag="rsc", name="rsc")
                    nc.vector.reciprocal(
                        rsc[:, 0:7],
                        ot[0][:, 0:455].rearrange("p (g c) -> p g c", c=65)[:, :, 64])
                    nc.vector.reciprocal(
                        rsc[:, 7:14],
                        ot[1][:, 0:455].rearrange("p (g c) -> p g c", c=65)[:, :, 64])
                    nc.vector.reciprocal(
                        rsc[:, 14:16],
                        ot[2][:, 0:130].rearrange("p (g c) -> p g c", c=65)[:, :, 64])
                    ts_(rsc[:, 8:16], rsc[:, 8:16], negl_t, None, OP.mult)
                    for qb in range(8):
                        b1, c1 = o_loc(0, qb)
                        b2, c2 = o_loc(1, qb)
                        ydst = yy[qb][:, 64 * h:64 * h + 64]
                        ts_(ydst, ot[b1][:, c1:c1 + 64],
                            rsc[:, qb:qb + 1], None, OP.mult)
                        stt(ydst, ot[b2][:, c2:c2 + 64],
                            rsc[:, 8 + qb:9 + qb], ydst, OP.mult, OP.add)

                # ---------- emission ----------
                # heads 0/1 q/k projections inline (needed before step 0)
                for h01 in (0, 1):
                    for u in qk_units(h01, "q"):
                        u()
                for h01 in (0, 1):
                    for u in qk_units(h01, "k"):
                        u()

                filler = []
                for j in range(NJ):
                    filler.extend(v_units(j))
                for q in range(NQ):
                    filler.extend(g_units(q))
                for h2 in range(2, 8):
                    filler.extend(qk_units(h2, "q"))
                    filler.extend(qk_units(h2, "k"))

                # software-pipelined attention loop: per step emit scores(i),
                # then o-matmuls of step i-1, then exp(i), then filler.  The
                # PE can run scores(i) during exp(i-1) (s pool bufs=2) and is
                # topped up with filler so it stays dense (p-state).
                steps = [(h, t, j) for h in range(8) for t in range(2)
                         for j in range(NJ)]
                otiles = {}
                prev = None

                def emit_o(ph, pt, pj, pex):
                    if pt == 0 and pj == 0:
                        otiles[ph] = [
                            ps_o.tile([128, 512], f32, tag=f"o{i}", name=f"o{i}")
                            for i in range(3)]
                    ot = otiles[ph]
                    for qb in range(NQ):
                        bi, c0 = o_loc(pt, qb)
                        # start only on the FIRST matmul into each bank this
                        # head: start_tensor_calc zeroes the whole bank, so
                        # later groups accumulate onto the zeroed regions.
                        first = (pj == 0) and (pt, qb) in ((0, 0), (0, 7), (1, 6))
                        nc.tensor.matmul(
                            ot[bi][:, c0:c0 + 65],
                            pex[:, 128 * qb:128 * (qb + 1)],
                            va[pj][:, ph, :],
                            start=first, stop=(pj == NJ - 1),
                            skip_group_check=True)
                    if pt == 1 and pj == NJ - 1:
                        combine(ph, ot)

                for idx, (h, t, j) in enumerate(steps):
                    s_t = ps_s.tile([128, S], f32, tag="s", name="s")
                    for n in range(2):
                        nc.tensor.matmul(
                            s_t[:, 512 * n:512 * (n + 1)],
                            kp[h][64 * t:64 * t + 64, 128 * j:128 * (j + 1)],
                            qp[h][64 * t:64 * t + 64, 512 * n:512 * (n + 1)],
                            start=True, stop=True)
                    if prev is not None:
                        emit_o(*prev)
                    ex = expp.tile([128, S], bf16, tag="ex", name="ex")
                    nc.scalar.activation(ex, s_t, AF.Exp, scale=INV)
                    prev = (h, t, j, ex)
                    npop = 2 if idx < 24 else 1
                    for _ in range(npop):
                        if filler:
                            filler.pop(0)()
                emit_o(*prev)
                while filler:
                    filler.pop(0)()

            # ---------- tail: GroupNorm stats + affine + gate + output ----
            with tc.tile_pool(name="ps_tail", bufs=1, space="PSUM") as ps_t, \
                 tc.tile_pool(name="y2p", bufs=2) as y2p, \
                 tc.tile_pool(name="oqp", bufs=3) as oqp, \
                 tc.tile_pool(name="tsb", bufs=1) as tsb:

                sy = ps_t.tile([1, 512], f32, tag="sy", name="sy")
                sy2 = ps_t.tile([1, 512], f32, tag="sy2", name="sy2")
                mcol = ps_t.tile([8, 2], f32, tag="mcol", name="mcol")
                rxA = ps_t.tile([1, 512], f32, tag="rxA", name="rxA")
                rxB = ps_t.tile([1, 512], f32, tag="rxB", name="rxB")
                ab = ps_t.tile([128, 1024], f32, tag="ab", name="ab")

                for qb in range(NQ):
                    ybf = y2p.tile([128, 512], bf16, tag="ybf", name="ybf")
                    nc.scalar.copy(ybf, yy[qb])
                    nc.tensor.matmul(sy, ones_c[:], ybf,
                                     start=(qb == 0), stop=(qb == NQ - 1))
                    y2 = y2p.tile([128, 512], bf16, tag="y2", name="y2")
                    if qb % 2 == 0:
                        nc.vector.tensor_mul(y2, yy[qb], yy[qb])
                    else:
                        nc.gpsimd.tensor_mul(y2, yy[qb], yy[qb])
                    nc.tensor.matmul(sy2, ones_c[:], y2,
                                     start=(qb == 0), stop=(qb == NQ - 1))

                gsum = tsb.tile([1, 8], f32, tag="gsum", name="gsum")
                g2 = tsb.tile([1, 8], f32, tag="g2", name="g2")
                nc.vector.tensor_reduce(
                    gsum, sy.rearrange("o (h g e) -> o g h e", h=8, g=8),
                    axis=AX.XY, op=OP.add)
                nc.vector.tensor_reduce(
                    g2, sy2.rearrange("o (h g e) -> o g h e", h=8, g=8),
                    axis=AX.XY, op=OP.add)
                nc.tensor.matmul(mcol[:, 0:1], gsum, one1, start=True, stop=False,
                                 skip_group_check=True)
                nc.tensor.matmul(mcol[:, 1:2], g2, one1, start=False, stop=True,
                                 skip_group_check=True)

                e2t = tsb.tile([8, 1], f32, tag="e2t", name="e2t")
                nm = tsb.tile([8, 1], f32, tag="nm", name="nm")
                veps = tsb.tile([8, 1], f32, tag="veps", name="veps")
                sd = tsb.tile([8, 1], f32, tag="sd", name="sd")
                rsd = tsb.tile([8, 1], f32, tag="rsd", name="rsd")
                rr = tsb.tile([8, 1], f32, tag="rr", name="rr")
                ts_(mrst[:, 1:2], mcol[:, 0:1], 1.0 / float(S * H * 8), None, OP.mult)
                ts_(e2t, mcol[:, 1:2], 1.0 / float(S * H * 8), None, OP.mult)
                ts_(nm, mrst[:, 1:2], mrst[:, 1:2], -1.0, OP.mult, OP.mult)
                stt(veps, nm, EPS, e2t, OP.add, OP.add)
                nc.scalar.activation(sd, veps, AF.Sqrt)
                nc.vector.reciprocal(rsd, sd)
                nc.vector.tensor_mul(rr, rsd, rsd)
                nc.vector.tensor_mul(rr, rr, veps)
                ts_(rr, rr, -0.5, 1.5, OP.mult, OP.add)
                nc.vector.tensor_mul(mrst[:, 0:1], rsd, rr)

                nc.tensor.matmul(rxA, mrst[:, 0:1], g8_t[:, :], start=True, stop=True)
                nc.tensor.matmul(rxB, mrst[:, 1:2], g8_t[:, :], start=True, stop=True)
                arow = tsb.tile([1, 512], f32, tag="arow", name="arow")
                btmp = tsb.tile([1, 512], f32, tag="btmp", name="btmp")
                brow = tsb.tile([1, 512], f32, tag="brow", name="brow")
                nc.vector.tensor_mul(arow, ghl_r, rxA)
                nc.vector.tensor_mul(btmp, rxB, arow)
                nc.vector.tensor_sub(brow, bhl_r, btmp)
                nc.tensor.matmul(ab[:, 0:512], ones_rf[:], arow,
                                 start=True, stop=True)
                nc.tensor.matmul(ab[:, 512:1024], ones_rf[:], brow,
                                 start=True, stop=True)

                ab_sb = tsb.tile([128, 1024], f32, tag="ab_sb", name="ab_sb")
                nc.vector.tensor_copy(ab_sb[:, 0:512], ab[:, 0:512])
                nc.vector.tensor_copy(ab_sb[:, 512:1024], ab[:, 512:1024])
                for qb in range(NQ):
                    oq = oqp.tile([128, 512], f32, tag="oq", name="oq")
                    if qb % 8 < 5:
                        nc.vector.tensor_mul(oq, yy[qb], ab_sb[:, 0:512])
                        nc.vector.tensor_add(oq, oq, ab_sb[:, 512:1024])
                        stt(oq, th[qb], 1.0, oq, OP.add, OP.mult)
                    else:
                        nc.gpsimd.tensor_mul(oq, yy[qb], ab_sb[:, 0:512])
                        nc.gpsimd.tensor_add(oq, oq, ab_sb[:, 512:1024])
                        stt(oq, th[qb], 1.0, oq, OP.add, OP.mult)
                    nc.sync.dma_start(out=out_d[128 * qb:128 * (qb + 1), :], in_=oq)

    nc.finalize()
    return nc


_CACHE = {}


def _get_nc():
    if "nc" not in _CACHE:
        _CACHE["nc"] = build_nc()
    return _CACHE["nc"]


def _host_prep(arrs):
    """Pack weights/biases into device layouts (bf16 x^T chunks etc.)."""
    from ml_dtypes import bfloat16 as bf

    def rpack(w):  # [512, C] -> [128, 4*C] with [p, C*r + c] = w[128r + p, c]
        c = w.shape[1]
        return np.ascontiguousarray(
            w.reshape(4, 128, c).transpose(1, 0, 2).reshape(128, 4 * c)).astype(bf)

    wq, wk, wv = arrs["Wq"], arrs["Wk"], arrs["Wv"]
    wg = np.ascontiguousarray(wq.reshape(DM, 8, 192)[:, :, 128:].reshape(DM, 512))
    bq, bk, bv = arrs["bq"], arrs["bk"], arrs["bv"]
    lam = float(arrs["lam"][0])
    li = float(arrs["lambda_init"][0])
    hl = 0.5 * (1.0 - li)

    bqk = np.zeros((128, 16), np.float32)
    for h in range(8):
        bqk[:, h] = bq[192 * h:192 * h + 128]
        bqk[:, 8 + h] = bk[128 * h:128 * h + 128]
    gb = bq.reshape(8, 192)[:, 128:].reshape(512)
    rows = np.concatenate([
        np.tile(arrs["gamma"], 8) * hl,
        np.tile(arrs["beta"], 8) * hl,
        gb, bv]).astype(np.float32).reshape(1, 2048)
    g8 = np.zeros((8, 512), np.float32)
    cols = np.arange(512)
    g8[(cols % 64) // 8, cols] = 1.0

    rowsb = np.concatenate([gb, bv]).astype(bf).reshape(1, 1024)
    shared = {
        "wq": rpack(wq), "wk": rpack(wk), "wv": rpack(wv), "wg": rpack(wg),
        "bqk": np.ascontiguousarray(bqk), "rows": rows, "rowsb": rowsb,
        "g8": g8, "negl": np.array([-lam], np.float32),
    }
    in_maps = []
    for i in range(B):
        m = dict(shared)
        for nm, key in (("xq", "query"), ("xk", "key"), ("xv", "values")):
            m[nm] = rpack(np.ascontiguousarray(arrs[key][i].T))
        in_maps.append(m)
    return in_maps


def run(inputs, trace=False, tmpdir=None):
    from concourse.bass_utils import run_bass_kernel_spmd
    nc = _get_nc()
    arrs = {k: np.asarray(v, dtype=np.float32) for k, v in inputs.items()}
    in_maps = _host_prep(arrs)
    res = run_bass_kernel_spmd(nc, in_maps, core_ids=list(range(B)),
                               trace=trace, tmpdir=tmpdir)
    out = np.stack([res.results[i]["out"] for i in range(B)], axis=0)
    return out.astype(np.float32), res


def kernel(**inputs):
    out, _ = run(inputs)
    return out


# revision 13
# speedup vs baseline: 1.1501x; 1.1501x over previous
# Differential multi-head attention (dual softmax + GroupNorm + sigmoid gating)
# for Trainium2, batch-parallel across 8 NeuronCores (one batch row per core).
#
# Per-core math (batch b):
#   q = query @ Wq + bq -> per head: q1, q2, gate (each S x 64)
#   k = key   @ Wk + bk -> per head: k1, k2
#   v = values@ Wv + bv -> per head: v (S x 64)
#   attn = softmax(q1 k1^T / 8) - lam * softmax(q2 k2^T / 8)
#   out  = GroupNorm_{8 groups over d, reduced over (S, heads, d-in-group)}(attn @ v)
#   out  = out * (1 - lambda_init) * sigmoid(gate)
#
# Layout strategy:
#  - Host pre-packs x^T and all weights as bf16 (layout marshalling only; the
#    math runs on device).  q/k projections are d-major (q1/q2 resp. k1/k2 in
#    complementary 64-partition halves of one [128,S] tile); score matmuls are
#    K=64 at PE row-tiles 0/64 (no zero padding needed).
#  - "Flipped" o-matmul: stationary = exp-score slice [128k x 128q], moving =
#    v_aug [128k x 65] -> out is q-major [128q, 65] with the exp row-sum in
#    column 64 (ones column in v_aug).  This halves PE time vs the d-major
#    o-matmul, makes softmax normalization a per-partition scalar multiply,
#    and leaves y in the exact output layout (no final transposes).
#  - GroupNorm stats via ones-column f32r matmuls (Sum y, Sum y^2 over seq on
#    partitions), group-reduced on DVE, rsqrt via sqrt+NR, then the per-column
#    affine A,B is expanded back to [128,512] with tiny K=1/K=8 matmuls.
#  - ACT runs (nearly) only the 128 [128,1024] exp instructions + 8 tanh; all
#    projection epilogues run on DVE/Pool.  PSUM: s double-buffered (4 banks),
#    o manually packed into 3 banks (7+7+2 groups of 65 cols), proj 1 bank.

import numpy as np

B, S, H, D = 8, 1024, 8, 64
DM = H * D  # 512
NJ = S // 128   # key 128-tiles
NQ = S // 128   # query 128-blocks
EPS = 1e-3
INV = 0.125
CNT = float(S * H)  # groupnorm count per group = S * H * (D//H) / ... = 1024*8


def build_nc():
    import concourse.bacc as bacc
    import concourse.tile as tile
    from concourse import mybir

    f32 = mybir.dt.float32
    f32r = mybir.dt.float32r
    bf16 = mybir.dt.bfloat16
    AF = mybir.ActivationFunctionType
    OP = mybir.AluOpType
    AX = mybir.AxisListType

    nc = bacc.Bacc(target_bir_lowering=False)
    xq_d = nc.dram_tensor("xq", [128, 4 * S], bf16, kind="ExternalInput")
    xk_d = nc.dram_tensor("xk", [128, 4 * S], bf16, kind="ExternalInput")
    xv_d = nc.dram_tensor("xv", [128, 4 * S], bf16, kind="ExternalInput")
    wq_d = nc.dram_tensor("wq", [128, 4 * 1536], bf16, kind="ExternalInput")
    wk_d = nc.dram_tensor("wk", [128, 4 * 1024], bf16, kind="ExternalInput")
    wv_d = nc.dram_tensor("wv", [128, 4 * 512], bf16, kind="ExternalInput")
    wg_d = nc.dram_tensor("wg", [128, 4 * 512], bf16, kind="ExternalInput")
    bqk_d = nc.dram_tensor("bqk", [128, 16], f32, kind="ExternalInput")
    rows_d = nc.dram_tensor("rows", [1, 2048], f32, kind="ExternalInput")
    rowsb_d = nc.dram_tensor("rowsb", [1, 1024], bf16, kind="ExternalInput")
    g8_d = nc.dram_tensor("g8", [8, 512], f32, kind="ExternalInput")
    negl_d = nc.dram_tensor("negl", [1], f32, kind="ExternalInput")
    out_d = nc.dram_tensor("out", [S, DM], f32, kind="ExternalOutput")

    ts_ = nc.vector.tensor_scalar
    stt = nc.vector.scalar_tensor_tensor
    gts_ = nc.gpsimd.tensor_scalar
    gstt = nc.gpsimd.scalar_tensor_tensor

    # (t, qb) accumulation group -> (o-bank index, column offset); 65 cols per
    # group (64 d + 1 ones-sum), packed 7 + 7 + 2 into three psum banks.
    def o_loc(t, qb):
        g = t * 8 + qb
        if g < 7:
            return 0, 65 * g
        if g < 14:
            return 1, 65 * (g - 7)
        return 2, 65 * (g - 14)

    with tile.TileContext(nc) as tc:
        with tc.tile_pool(name="persist", bufs=1) as pp:
            # ---------- persistent SBUF ----------
            xq_t = pp.tile([128, 4 * S], bf16, tag="xq_t", name="xq_t")
            xk_t = pp.tile([128, 4 * S], bf16, tag="xk_t", name="xk_t")
            xv_t = pp.tile([128, 4 * S], bf16, tag="xv_t", name="xv_t")
            wq_t = pp.tile([128, 4 * 1536], bf16, tag="wq_t", name="wq_t")
            wk_t = pp.tile([128, 4 * 1024], bf16, tag="wk_t", name="wk_t")
            wv_t = pp.tile([128, 4 * 512], bf16, tag="wv_t", name="wv_t")
            wg_t = pp.tile([128, 4 * 512], bf16, tag="wg_t", name="wg_t")
            bqk_t = pp.tile([128, 16], f32, tag="bqk_t", name="bqk_t")
            rows_t = pp.tile([1, 2048], f32, tag="rows_t", name="rows_t")
            rowsb_t = pp.tile([1, 1024], bf16, tag="rowsb_t", name="rowsb_t")
            g8_t = pp.tile([8, 512], f32, tag="g8_t", name="g8_t")
            negl_t = pp.tile([128, 1], f32, tag="negl_t", name="negl_t")
            ones_c = pp.tile([128, 1], bf16, tag="ones_c", name="ones_c")
            ones_r = pp.tile([1, 128], bf16, tag="ones_r", name="ones_r")
            ones_rf = pp.tile([1, 128], f32, tag="ones_rf", name="ones_rf")
            one1 = pp.tile([1, 1], f32, tag="one1", name="one1")
            qp = [pp.tile([128, S], bf16, tag=f"qp{h}", name=f"qp{h}") for h in range(8)]
            kp = [pp.tile([128, S], bf16, tag=f"kp{h}", name=f"kp{h}") for h in range(8)]
            va = [pp.tile([128, 8, 65], bf16, tag=f"va{j}", name=f"va{j}") for j in range(NJ)]
            th = [pp.tile([128, 512], f32, tag=f"th{q}", name=f"th{q}") for q in range(NQ)]
            yy = [pp.tile([128, 512], f32, tag=f"yy{q}", name=f"yy{q}") for q in range(NQ)]
            mrst = pp.tile([8, 2], f32, tag="mrst", name="mrst")

            # ---------- DMAs ----------
            # SP queue: x tensors, n-half-first so the first score matmuls
            # can start as soon as ~1MB has landed.  ACT queue: weights,
            # heads-0/1 slices of wq/wk prefetched ahead of the bulk.
            xqv = xq_t.rearrange("p (r s) -> p r s", s=S)
            xkv = xk_t.rearrange("p (r s) -> p r s", s=S)
            xqd = xq_d[:, :].rearrange("p (r s) -> p r s", s=S)
            xkd = xk_d[:, :].rearrange("p (r s) -> p r s", s=S)
            nc.sync.dma_start(out=xqv[:, :, 0:512], in_=xqd[:, :, 0:512])
            nc.sync.dma_start(out=xkv[:, :, 0:512], in_=xkd[:, :, 0:512])
            nc.sync.dma_start(out=xqv[:, :, 512:1024], in_=xqd[:, :, 512:1024])
            nc.sync.dma_start(out=xkv[:, :, 512:1024], in_=xkd[:, :, 512:1024])
            nc.sync.dma_start(out=xv_t, in_=xv_d[:, :])
            wqv = wq_t.rearrange("p (r c) -> p r c", c=1536)
            wkv = wk_t.rearrange("p (r c) -> p r c", c=1024)
            wqd = wq_d[:, :].rearrange("p (r c) -> p r c", c=1536)
            wkd = wk_d[:, :].rearrange("p (r c) -> p r c", c=1024)
            nc.scalar.dma_start(out=wqv[:, :, 0:384], in_=wqd[:, :, 0:384])
            nc.scalar.dma_start(out=wkv[:, :, 0:256], in_=wkd[:, :, 0:256])
            nc.scalar.dma_start(out=wqv[:, :, 384:1536], in_=wqd[:, :, 384:1536])
            nc.scalar.dma_start(out=wkv[:, :, 256:1024], in_=wkd[:, :, 256:1024])
            nc.scalar.dma_start(out=wv_t, in_=wv_d[:, :])
            nc.scalar.dma_start(out=wg_t, in_=wg_d[:, :])
            nc.gpsimd.dma_start(out=bqk_t, in_=bqk_d[:, :])
            nc.gpsimd.dma_start(out=rows_t, in_=rows_d[:, :])
            nc.gpsimd.dma_start(out=rowsb_t, in_=rowsb_d[:, :])
            nc.gpsimd.dma_start(out=g8_t, in_=g8_d[:, :])
            nc.gpsimd.dma_start(out=negl_t, in_=negl_d[:].to_broadcast([128, 1]))
            nc.gpsimd.memset(ones_c, 1.0)
            nc.gpsimd.memset(ones_r, 1.0)
            nc.gpsimd.memset(ones_rf, 1.0)
            nc.gpsimd.memset(one1, 1.0)
            for j in range(NJ):
                nc.gpsimd.memset(va[j][:, :, 64:65], 1.0)

            ghl_r = rows_t[:, 0:512]
            bhl_r = rows_t[:, 512:1024]
            gb_r = rowsb_t[:, 0:512]
            vb_r = rowsb_t[:, 512:1024]

            with tc.tile_pool(name="ps_proj", bufs=1, space="PSUM") as ps_proj, \
                 tc.tile_pool(name="ps_s", bufs=2, space="PSUM") as ps_s, \
                 tc.tile_pool(name="ps_o", bufs=1, space="PSUM") as ps_o, \
                 tc.tile_pool(name="expp", bufs=3) as expp, \
                 tc.tile_pool(name="rscp", bufs=2) as rscp:

                # ---------- projection filler units (one PE matmul each) ----
                # Projections for heads >= 2, v, and gate are emitted as
                # single-matmul units interleaved into the attention loop so
                # the PE stream stays dense while the loop is ACT(exp)-paced.
                def qk_units(h, kind):
                    if kind == "q":
                        w_t, x_t, dst = wq_t, xq_t, qp[h]
                        bcol = bqk_t[:, h:h + 1]
                        wofs = lambda r: 1536 * r + 192 * h
                    else:
                        w_t, x_t, dst = wk_t, xk_t, kp[h]
                        bcol = bqk_t[:, 8 + h:9 + h]
                        wofs = lambda r: 1024 * r + 128 * h
                    st = {}
                    units = []
                    for n in range(2):
                        for r in range(4):
                            def f(n=n, r=r):
                                if r == 0:
                                    st[n] = ps_proj.tile(
                                        [128, 512], f32, tag="proj", name="proj")
                                nc.tensor.matmul(
                                    st[n], w_t[:, wofs(r):wofs(r) + 128],
                                    x_t[:, S * r + 512 * n:S * r + 512 * (n + 1)],
                                    start=(r == 0), stop=(r == 3))
                                if r == 3:
                                    ts_(dst[:, 512 * n:512 * (n + 1)], st[n],
                                        bcol, None, OP.add)
                            units.append(f)
                    return units

                def vg_units(q, kind):
                    # v projection (j-tile q) or gate projection (q-block q)
                    st = {}
                    units = []
                    for r in range(4):
                        def f(r=r):
                            x_t = xv_t if kind == "v" else xq_t
                            w_t = wv_t if kind == "v" else wg_t
                            if r == 0:
                                st[0] = ps_proj.tile(
                                    [128, 512], f32, tag="proj", name="proj")
                            nc.tensor.matmul(
                                st[0], x_t[:, S * r + 128 * q:S * r + 128 * (q + 1)],
                                w_t[:, 512 * r:512 * (r + 1)],
                                start=(r == 0), stop=False)
                            if r == 3:
                                if kind == "v":
                                    nc.tensor.matmul(st[0], ones_r[:], vb_r,
                                                     start=False, stop=True)
                                    nc.vector.tensor_copy(
                                        va[q][:, :, 0:64],
                                        st[0].rearrange("p (h d) -> p h d", d=64))
                                else:
                                    nc.tensor.matmul(st[0], ones_r[:], gb_r,
                                                     start=False, stop=True)
                                    nc.scalar.activation(
                                        th[q], st[0], AF.Tanh, scale=0.5)
                        units.append(f)
                    return units

                # ---------- combine: y[:, 64h:64h+64] = o1*r1 - lam*o2*r2 --
                def combine(h, ot):
                    rsc = rscp.tile([128, 16], f32, tag="rsc", name="rsc")
                    nc.vector.reciprocal(
                        rsc[:, 0:7],
                        ot[0][:, 0:455].rearrange("p (g c) -> p g c", c=65)[:, :, 64])
                    nc.vector.reciprocal(
                        rsc[:, 7:14],
                        ot[1][:, 0:455].rearrange("p (g c) -> p g c", c=65)[:, :, 64])
                    nc.vector.reciprocal(
                        rsc[:, 14:16],
                        ot[2][:, 0:130].rearrange("p (g c) -> p g c", c=65)[:, :, 64])
                    ts_(rsc[:, 8:16], rsc[:, 8:16], negl_t, None, OP.mult)
                    for qb in range(8):
                        b1, c1 = o_loc(0, qb)
                        b2, c2 = o_loc(1, qb)
                        ydst = yy[qb][:, 64 * h:64 * h + 64]
                        ts_(ydst, ot[b1][:, c1:c1 + 64],
                            rsc[:, qb:qb + 1], None, OP.mult)
                        stt(ydst, ot[b2][:, c2:c2 + 64],
                            rsc[:, 8 + qb:9 + qb], ydst, OP.mult, OP.add)

                # ---------- emission ----------
                # heads 0/1 q/k projections inline via the s-pool tiles
                # ([128,1024] psum, single wide epilogue) for a dense start.
                for h01, kind in ((0, "q"), (1, "q"), (0, "k"), (1, "k")):
                    w_t = wq_t if kind == "q" else wk_t
                    x_t = xq_t if kind == "q" else xk_t
                    dst = qp[h01] if kind == "q" else kp[h01]
                    bcol = bqk_t[:, h01:h01 + 1] if kind == "q" \
                        else bqk_t[:, 8 + h01:9 + h01]
                    wofs = (lambda r: 1536 * r + 192 * h01) if kind == "q" \
                        else (lambda r: 1024 * r + 128 * h01)
                    ps = ps_s.tile([128, S], f32, tag="s", name="s")
                    for n in range(2):
                        for r in range(4):
                            nc.tensor.matmul(
                                ps[:, 512 * n:512 * (n + 1)],
                                w_t[:, wofs(r):wofs(r) + 128],
                                x_t[:, S * r + 512 * n:S * r + 512 * (n + 1)],
                                start=(r == 0), stop=(r == 3))
                    ts_(dst, ps, bcol, None, OP.add)

                filler = []
                for j in range(NJ):
                    filler.extend(vg_units(j, "v"))
                for q in range(NQ):
                    filler.extend(vg_units(q, "g"))
                for h2 in range(2, 8):
                    filler.extend(qk_units(h2, "q"))
                    filler.extend(qk_units(h2, "k"))

                # software-pipelined attention loop with a 2-step o-lag:
                # everything PE does in step i depends only on exp(i-2), so
                # the PE never waits mid-step and the exp stream stays dense.
                steps = [(h, t, j) for h in range(8) for t in range(2)
                         for j in range(NJ)]
                otiles = {}
                pending = []

                def emit_o(ph, pt, pj, pex):
                    if pt == 0 and pj == 0:
                        otiles[ph] = [
                            ps_o.tile([128, 512], f32, tag=f"o{i}", name=f"o{i}")
                            for i in range(3)]
                    ot = otiles[ph]
                    for qb in range(NQ):
                        bi, c0 = o_loc(pt, qb)
                        # start only on the FIRST matmul into each bank this
                        # head: start_tensor_calc zeroes the whole bank, so
                        # later groups accumulate onto the zeroed regions.
                        first = (pj == 0) and (pt, qb) in ((0, 0), (0, 7), (1, 6))
                        nc.tensor.matmul(
                            ot[bi][:, c0:c0 + 65],
                            pex[:, 128 * qb:128 * (qb + 1)],
                            va[pj][:, ph, :],
                            start=first, stop=(pj == NJ - 1),
                            skip_group_check=True)
                    if pt == 1 and pj == NJ - 1:
                        combine(ph, ot)

                for idx, (h, t, j) in enumerate(steps):
                    s_t = ps_s.tile([128, S], f32, tag="s", name="s")
                    for n in range(2):
                        nc.tensor.matmul(
                            s_t[:, 512 * n:512 * (n + 1)],
                            kp[h][64 * t:64 * t + 64, 128 * j:128 * (j + 1)],
                            qp[h][64 * t:64 * t + 64, 512 * n:512 * (n + 1)],
                            start=True, stop=True)
                    if len(pending) >= 2:
                        emit_o(*pending.pop(0))
                    ex = expp.tile([128, S], bf16, tag="ex", name="ex")
                    nc.scalar.activation(ex, s_t, AF.Exp, scale=INV)
                    pending.append((h, t, j, ex))
                    npop = 4 if idx < 10 else (2 if idx < 48 else 1)
                    for _ in range(npop):
                        if filler:
                            filler.pop(0)()
                while pending:
                    emit_o(*pending.pop(0))
                while filler:
                    filler.pop(0)()

            # ---------- tail: GroupNorm stats + affine + gate + output ----
            with tc.tile_pool(name="ps_tail", bufs=1, space="PSUM") as ps_t, \
                 tc.tile_pool(name="y2p", bufs=2) as y2p, \
                 tc.tile_pool(name="oqp", bufs=3) as oqp, \
                 tc.tile_pool(name="tsb", bufs=1) as tsb:

                sy = ps_t.tile([1, 512], f32, tag="sy", name="sy")
                sy2 = ps_t.tile([1, 512], f32, tag="sy2", name="sy2")
                mcol = ps_t.tile([8, 2], f32, tag="mcol", name="mcol")
                rxA = ps_t.tile([1, 512], f32, tag="rxA", name="rxA")
                rxB = ps_t.tile([1, 512], f32, tag="rxB", name="rxB")
                ab = ps_t.tile([128, 1024], f32, tag="ab", name="ab")

                for qb in range(NQ):
                    ybf = y2p.tile([128, 512], bf16, tag="ybf", name="ybf")
                    nc.scalar.copy(ybf, yy[qb])
                    nc.tensor.matmul(sy, ones_c[:], ybf,
                                     start=(qb == 0), stop=(qb == NQ - 1))
                    y2 = y2p.tile([128, 512], bf16, tag="y2", name="y2")
                    if qb % 2 == 0:
                        nc.vector.tensor_mul(y2, yy[qb], yy[qb])
                    else:
                        nc.gpsimd.tensor_mul(y2, yy[qb], yy[qb])
                    nc.tensor.matmul(sy2, ones_c[:], y2,
                                     start=(qb == 0), stop=(qb == NQ - 1))

                gsum = tsb.tile([1, 8], f32, tag="gsum", name="gsum")
                g2 = tsb.tile([1, 8], f32, tag="g2", name="g2")
                nc.vector.tensor_reduce(
                    gsum, sy.rearrange("o (h g e) -> o g h e", h=8, g=8),
                    axis=AX.XY, op=OP.add)
                nc.vector.tensor_reduce(
                    g2, sy2.rearrange("o (h g e) -> o g h e", h=8, g=8),
                    axis=AX.XY, op=OP.add)
                nc.tensor.matmul(mcol[:, 0:1], gsum, one1, start=True, stop=False,
                                 skip_group_check=True)
                nc.tensor.matmul(mcol[:, 1:2], g2, one1, start=False, stop=True,
                                 skip_group_check=True)

                e2t = tsb.tile([8, 1], f32, tag="e2t", name="e2t")
                nm = tsb.tile([8, 1], f32, tag="nm", name="nm")
                veps = tsb.tile([8, 1], f32, tag="veps", name="veps")
                sd = tsb.tile([8, 1], f32, tag="sd", name="sd")
                rsd = tsb.tile([8, 1], f32, tag="rsd", name="rsd")
                rr = tsb.tile([8, 1], f32, tag="rr", name="rr")
                ts_(mrst[:, 1:2], mcol[:, 0:1], 1.0 / float(S * H * 8), None, OP.mult)
                ts_(e2t, mcol[:, 1:2], 1.0 / float(S * H * 8), None, OP.mult)
                ts_(nm, mrst[:, 1:2], mrst[:, 1:2], -1.0, OP.mult, OP.mult)
                stt(veps, nm, EPS, e2t, OP.add, OP.add)
                nc.scalar.activation(sd, veps, AF.Sqrt)
                nc.vector.reciprocal(rsd, sd)
                nc.vector.tensor_mul(rr, rsd, rsd)
                nc.vector.tensor_mul(rr, rr, veps)
                ts_(rr, rr, -0.5, 1.5, OP.mult, OP.add)
                nc.vector.tensor_mul(mrst[:, 0:1], rsd, rr)

                nc.tensor.matmul(rxA, mrst[:, 0:1], g8_t[:, :], start=True, stop=True)
                nc.tensor.matmul(rxB, mrst[:, 1:2], g8_t[:, :], start=True, stop=True)
                arow = tsb.tile([1, 512], f32, tag="arow", name="arow")
                btmp = tsb.tile([1, 512], f32, tag="btmp", name="btmp")
                brow = tsb.tile([1, 512], f32, tag="brow", name="brow")
                nc.vector.tensor_mul(arow, ghl_r, rxA)
                nc.vector.tensor_mul(btmp, rxB, arow)
                nc.vector.tensor_sub(brow, bhl_r, btmp)
                nc.tensor.matmul(ab[:, 0:512], ones_rf[:], arow,
                                 start=True, stop=True)
                nc.tensor.matmul(ab[:, 512:1024], ones_rf[:], brow,
                                 start=True, stop=True)

                ab_sb = tsb.tile([128, 1024], f32, tag="ab_sb", name="ab_sb")
                nc.vector.tensor_copy(ab_sb[:, 0:512], ab[:, 0:512])
                nc.vector.tensor_copy(ab_sb[:, 512:1024], ab[:, 512:1024])
                for qb in range(NQ):
                    oq = oqp.tile([128, 512], f32, tag="oq", name="oq")
                    if qb % 8 < 5:
                        nc.vector.tensor_mul(oq, yy[qb], ab_sb[:, 0:512])
                        nc.vector.tensor_add(oq, oq, ab_sb[:, 512:1024])
                        stt(oq, th[qb], 1.0, oq, OP.add, OP.mult)
                    else:
                        nc.gpsimd.tensor_mul(oq, yy[qb], ab_sb[:, 0:512])
                        nc.gpsimd.tensor_add(oq, oq, ab_sb[:, 512:1024])
                        stt(oq, th[qb], 1.0, oq, OP.add, OP.mult)
                    nc.sync.dma_start(out=out_d[128 * qb:128 * (qb + 1), :], in_=oq)

    nc.finalize()
    return nc


_CACHE = {}


def _get_nc():
    if "nc" not in _CACHE:
        _CACHE["nc"] = build_nc()
    return _CACHE["nc"]


def _host_prep(arrs):
    """Pack weights/biases into device layouts (bf16 x^T chunks etc.)."""
    from ml_dtypes import bfloat16 as bf

    def rpack(w):  # [512, C] -> [128, 4*C] with [p, C*r + c] = w[128r + p, c]
        c = w.shape[1]
        return np.ascontiguousarray(
            w.reshape(4, 128, c).transpose(1, 0, 2).reshape(128, 4 * c)).astype(bf)

    wq, wk, wv = arrs["Wq"], arrs["Wk"], arrs["Wv"]
    wg = np.ascontiguousarray(wq.reshape(DM, 8, 192)[:, :, 128:].reshape(DM, 512))
    bq, bk, bv = arrs["bq"], arrs["bk"], arrs["bv"]
    lam = float(arrs["lam"][0])
    li = float(arrs["lambda_init"][0])
    hl = 0.5 * (1.0 - li)

    bqk = np.zeros((128, 16), np.float32)
    for h in range(8):
        bqk[:, h] = bq[192 * h:192 * h + 128]
        bqk[:, 8 + h] = bk[128 * h:128 * h + 128]
    gb = bq.reshape(8, 192)[:, 128:].reshape(512)
    rows = np.concatenate([
        np.tile(arrs["gamma"], 8) * hl,
        np.tile(arrs["beta"], 8) * hl,
        gb, bv]).astype(np.float32).reshape(1, 2048)
    g8 = np.zeros((8, 512), np.float32)
    cols = np.arange(512)
    g8[(cols % 64) // 8, cols] = 1.0

    rowsb = np.concatenate([gb, bv]).astype(bf).reshape(1, 1024)
    shared = {
        "wq": rpack(wq), "wk": rpack(wk), "wv": rpack(wv), "wg": rpack(wg),
        "bqk": np.ascontiguousarray(bqk), "rows": rows, "rowsb": rowsb,
        "g8": g8, "negl": np.array([-lam], np.float32),
    }
    in_maps = []
    for i in range(B):
        m = dict(shared)
        for nm, key in (("xq", "query"), ("xk", "key"), ("xv", "values")):
            m[nm] = rpack(np.ascontiguousarray(arrs[key][i].T))
        in_maps.append(m)
    return in_maps


def run(inputs, trace=False, tmpdir=None):
    from concourse.bass_utils import run_bass_kernel_spmd
    nc = _get_nc()
    arrs = {k: np.asarray(v, dtype=np.float32) for k, v in inputs.items()}
    in_maps = _host_prep(arrs)
    res = run_bass_kernel_spmd(nc, in_maps, core_ids=list(range(B)),
                               trace=trace, tmpdir=tmpdir)
    out = np.stack([res.results[i]["out"] for i in range(B)], axis=0)
    return out.astype(np.float32), res


def kernel(**inputs):
    out, _ = run(inputs)
    return out


# revision 14
# speedup vs baseline: 1.1964x; 1.0402x over previous
# Differential multi-head attention (dual softmax + GroupNorm + sigmoid gating)
# for Trainium2, batch-parallel across 8 NeuronCores (one batch row per core).
#
# Per-core math (batch b):
#   q = query @ Wq + bq -> per head: q1, q2, gate (each S x 64)
#   k = key   @ Wk + bk -> per head: k1, k2
#   v = values@ Wv + bv -> per head: v (S x 64)
#   attn = softmax(q1 k1^T / 8) - lam * softmax(q2 k2^T / 8)
#   out  = GroupNorm_{8 groups over d, reduced over (S, heads, d-in-group)}(attn @ v)
#   out  = out * (1 - lambda_init) * sigmoid(gate)
#
# Layout strategy:
#  - Host pre-packs x^T and all weights as bf16 (layout marshalling only; the
#    math runs on device).  q/k projections are d-major (q1/q2 resp. k1/k2 in
#    complementary 64-partition halves of one [128,S] tile); score matmuls are
#    K=64 at PE row-tiles 0/64 (no zero padding needed).
#  - "Flipped" o-matmul: stationary = exp-score slice [128k x 128q], moving =
#    v_aug [128k x 65] -> out is q-major [128q, 65] with the exp row-sum in
#    column 64 (ones column in v_aug).  This halves PE time vs the d-major
#    o-matmul, makes softmax normalization a per-partition scalar multiply,
#    and leaves y in the exact output layout (no final transposes).
#  - GroupNorm stats via ones-column f32r matmuls (Sum y, Sum y^2 over seq on
#    partitions), group-reduced on DVE, rsqrt via sqrt+NR, then the per-column
#    affine A,B is expanded back to [128,512] with tiny K=1/K=8 matmuls.
#  - ACT runs (nearly) only the 128 [128,1024] exp instructions + 8 tanh; all
#    projection epilogues run on DVE/Pool.  PSUM: s double-buffered (4 banks),
#    o manually packed into 3 banks (7+7+2 groups of 65 cols), proj 1 bank.

import numpy as np

B, S, H, D = 8, 1024, 8, 64
DM = H * D  # 512
NJ = S // 128   # key 128-tiles
NQ = S // 128   # query 128-blocks
EPS = 1e-3
INV = 0.125
CNT = float(S * H)  # groupnorm count per group = S * H * (D//H) / ... = 1024*8


def build_nc():
    import concourse.bacc as bacc
    import concourse.tile as tile
    from concourse import mybir

    f32 = mybir.dt.float32
    f32r = mybir.dt.float32r
    bf16 = mybir.dt.bfloat16
    AF = mybir.ActivationFunctionType
    OP = mybir.AluOpType
    AX = mybir.AxisListType

    nc = bacc.Bacc(target_bir_lowering=False)
    xq_d = nc.dram_tensor("xq", [128, 4 * S], bf16, kind="ExternalInput")
    xk_d = nc.dram_tensor("xk", [128, 4 * S], bf16, kind="ExternalInput")
    xv_d = nc.dram_tensor("xv", [128, 4 * S], bf16, kind="ExternalInput")
    wq_d = nc.dram_tensor("wq", [128, 4 * 1536], bf16, kind="ExternalInput")
    wk_d = nc.dram_tensor("wk", [128, 4 * 1024], bf16, kind="ExternalInput")
    wv_d = nc.dram_tensor("wv", [128, 4 * 512], bf16, kind="ExternalInput")
    wg_d = nc.dram_tensor("wg", [128, 4 * 512], bf16, kind="ExternalInput")
    bqk_d = nc.dram_tensor("bqk", [128, 16], f32, kind="ExternalInput")
    rows_d = nc.dram_tensor("rows", [1, 2048], f32, kind="ExternalInput")
    rowsb_d = nc.dram_tensor("rowsb", [1, 1024], bf16, kind="ExternalInput")
    g8_d = nc.dram_tensor("g8", [8, 512], f32, kind="ExternalInput")
    negl_d = nc.dram_tensor("negl", [1], f32, kind="ExternalInput")
    out_d = nc.dram_tensor("out", [S, DM], f32, kind="ExternalOutput")

    ts_ = nc.vector.tensor_scalar
    stt = nc.vector.scalar_tensor_tensor
    gts_ = nc.gpsimd.tensor_scalar
    gstt = nc.gpsimd.scalar_tensor_tensor

    # (t, qb) accumulation group -> (o-bank index, column offset); 65 cols per
    # group (64 d + 1 ones-sum), packed 7 + 7 + 2 into three psum banks.
    def o_loc(t, qb):
        g = t * 8 + qb
        if g < 7:
            return 0, 65 * g
        if g < 14:
            return 1, 65 * (g - 7)
        return 2, 65 * (g - 14)

    with tile.TileContext(nc) as tc:
        with tc.tile_pool(name="persist", bufs=1) as pp:
            # ---------- persistent SBUF ----------
            xq_t = pp.tile([128, 4 * S], bf16, tag="xq_t", name="xq_t")
            xk_t = pp.tile([128, 4 * S], bf16, tag="xk_t", name="xk_t")
            xv_t = pp.tile([128, 4 * S], bf16, tag="xv_t", name="xv_t")
            wq_t = pp.tile([128, 4 * 1536], bf16, tag="wq_t", name="wq_t")
            wk_t = pp.tile([128, 4 * 1024], bf16, tag="wk_t", name="wk_t")
            wv_t = pp.tile([128, 4 * 512], bf16, tag="wv_t", name="wv_t")
            wg_t = pp.tile([128, 4 * 512], bf16, tag="wg_t", name="wg_t")
            bqk_t = pp.tile([128, 16], f32, tag="bqk_t", name="bqk_t")
            rows_t = pp.tile([1, 2048], f32, tag="rows_t", name="rows_t")
            rowsb_t = pp.tile([1, 1024], bf16, tag="rowsb_t", name="rowsb_t")
            g8_t = pp.tile([8, 512], f32, tag="g8_t", name="g8_t")
            negl_t = pp.tile([128, 1], f32, tag="negl_t", name="negl_t")
            ones_c = pp.tile([128, 1], bf16, tag="ones_c", name="ones_c")
            ones_r = pp.tile([1, 128], bf16, tag="ones_r", name="ones_r")
            ones_rf = pp.tile([1, 128], f32, tag="ones_rf", name="ones_rf")
            one1 = pp.tile([1, 1], f32, tag="one1", name="one1")
            qp = [pp.tile([128, S], bf16, tag=f"qp{h}", name=f"qp{h}") for h in range(8)]
            kp = [pp.tile([128, S], bf16, tag=f"kp{h}", name=f"kp{h}") for h in range(8)]
            va = [pp.tile([128, 8, 65], bf16, tag=f"va{j}", name=f"va{j}") for j in range(NJ)]
            th = [pp.tile([128, 512], f32, tag=f"th{q}", name=f"th{q}") for q in range(NQ)]
            yy = [pp.tile([128, 512], f32, tag=f"yy{q}", name=f"yy{q}") for q in range(NQ)]
            mrst = pp.tile([8, 2], f32, tag="mrst", name="mrst")

            # ---------- DMAs ----------
            # SP queue: x tensors, n-half-first so the first score matmuls
            # can start as soon as ~1MB has landed.  ACT queue: weights,
            # heads-0/1 slices of wq/wk prefetched ahead of the bulk.
            xqv = xq_t.rearrange("p (r s) -> p r s", s=S)
            xkv = xk_t.rearrange("p (r s) -> p r s", s=S)
            xqd = xq_d[:, :].rearrange("p (r s) -> p r s", s=S)
            xkd = xk_d[:, :].rearrange("p (r s) -> p r s", s=S)
            nc.sync.dma_start(out=xqv[:, :, 0:512], in_=xqd[:, :, 0:512])
            nc.sync.dma_start(out=xkv[:, :, 0:512], in_=xkd[:, :, 0:512])
            nc.sync.dma_start(out=xqv[:, :, 512:1024], in_=xqd[:, :, 512:1024])
            nc.sync.dma_start(out=xkv[:, :, 512:1024], in_=xkd[:, :, 512:1024])
            nc.sync.dma_start(out=xv_t, in_=xv_d[:, :])
            wqv = wq_t.rearrange("p (r c) -> p r c", c=1536)
            wkv = wk_t.rearrange("p (r c) -> p r c", c=1024)
            wqd = wq_d[:, :].rearrange("p (r c) -> p r c", c=1536)
            wkd = wk_d[:, :].rearrange("p (r c) -> p r c", c=1024)
            nc.scalar.dma_start(out=wqv[:, :, 0:384], in_=wqd[:, :, 0:384])
            nc.scalar.dma_start(out=wkv[:, :, 0:256], in_=wkd[:, :, 0:256])
            nc.scalar.dma_start(out=wv_t, in_=wv_d[:, :])
            nc.scalar.dma_start(out=wg_t, in_=wg_d[:, :])
            nc.scalar.dma_start(out=wqv[:, :, 384:1536], in_=wqd[:, :, 384:1536])
            nc.scalar.dma_start(out=wkv[:, :, 256:1024], in_=wkd[:, :, 256:1024])
            nc.gpsimd.dma_start(out=bqk_t, in_=bqk_d[:, :])
            nc.gpsimd.dma_start(out=rows_t, in_=rows_d[:, :])
            nc.gpsimd.dma_start(out=rowsb_t, in_=rowsb_d[:, :])
            nc.gpsimd.dma_start(out=g8_t, in_=g8_d[:, :])
            nc.gpsimd.dma_start(out=negl_t, in_=negl_d[:].to_broadcast([128, 1]))
            nc.gpsimd.memset(ones_c, 1.0)
            nc.gpsimd.memset(ones_r, 1.0)
            nc.gpsimd.memset(ones_rf, 1.0)
            nc.gpsimd.memset(one1, 1.0)
            for j in range(NJ):
                nc.gpsimd.memset(va[j][:, :, 64:65], 1.0)

            ghl_r = rows_t[:, 0:512]
            bhl_r = rows_t[:, 512:1024]
            gb_r = rowsb_t[:, 0:512]
            vb_r = rowsb_t[:, 512:1024]

            with tc.tile_pool(name="ps_proj", bufs=1, space="PSUM") as ps_proj, \
                 tc.tile_pool(name="ps_s", bufs=2, space="PSUM") as ps_s, \
                 tc.tile_pool(name="ps_o", bufs=1, space="PSUM") as ps_o, \
                 tc.tile_pool(name="expp", bufs=3) as expp, \
                 tc.tile_pool(name="rscp", bufs=2) as rscp:

                # ---------- projection filler units (one PE matmul each) ----
                # Projections for heads >= 2, v, and gate are emitted as
                # single-matmul units interleaved into the attention loop so
                # the PE stream stays dense while the loop is ACT(exp)-paced.
                def qk_units(h, kind):
                    if kind == "q":
                        w_t, x_t, dst = wq_t, xq_t, qp[h]
                        bcol = bqk_t[:, h:h + 1]
                        wofs = lambda r: 1536 * r + 192 * h
                    else:
                        w_t, x_t, dst = wk_t, xk_t, kp[h]
                        bcol = bqk_t[:, 8 + h:9 + h]
                        wofs = lambda r: 1024 * r + 128 * h
                    st = {}
                    units = []
                    for n in range(2):
                        for r in range(4):
                            def f(n=n, r=r):
                                if r == 0:
                                    st[n] = ps_proj.tile(
                                        [128, 512], f32, tag="proj", name="proj")
                                nc.tensor.matmul(
                                    st[n], w_t[:, wofs(r):wofs(r) + 128],
                                    x_t[:, S * r + 512 * n:S * r + 512 * (n + 1)],
                                    start=(r == 0), stop=(r == 3))
                                if r == 3:
                                    ts_(dst[:, 512 * n:512 * (n + 1)], st[n],
                                        bcol, None, OP.add)
                            units.append(f)
                    return units

                def vg_units(q, kind):
                    # v projection (j-tile q) or gate projection (q-block q)
                    st = {}
                    units = []
                    for r in range(4):
                        def f(r=r):
                            x_t = xv_t if kind == "v" else xq_t
                            w_t = wv_t if kind == "v" else wg_t
                            if r == 0:
                                st[0] = ps_proj.tile(
                                    [128, 512], f32, tag="proj", name="proj")
                            nc.tensor.matmul(
                                st[0], x_t[:, S * r + 128 * q:S * r + 128 * (q + 1)],
                                w_t[:, 512 * r:512 * (r + 1)],
                                start=(r == 0), stop=False)
                            if r == 3:
                                if kind == "v":
                                    nc.tensor.matmul(st[0], ones_r[:], vb_r,
                                                     start=False, stop=True)
                                    nc.vector.tensor_copy(
                                        va[q][:, :, 0:64],
                                        st[0].rearrange("p (h d) -> p h d", d=64))
                                else:
                                    nc.tensor.matmul(st[0], ones_r[:], gb_r,
                                                     start=False, stop=True)
                                    nc.scalar.activation(
                                        th[q], st[0], AF.Tanh, scale=0.5)
                        units.append(f)
                    return units

                # ---------- combine: y[:, 64h:64h+64] = o1*r1 - lam*o2*r2 --
                def combine(h, ot):
                    rsc = rscp.tile([128, 16], f32, tag="rsc", name="rsc")
                    nc.vector.reciprocal(
                        rsc[:, 0:7],
                        ot[0][:, 0:455].rearrange("p (g c) -> p g c", c=65)[:, :, 64])
                    nc.vector.reciprocal(
                        rsc[:, 7:14],
                        ot[1][:, 0:455].rearrange("p (g c) -> p g c", c=65)[:, :, 64])
                    nc.vector.reciprocal(
                        rsc[:, 14:16],
                        ot[2][:, 0:130].rearrange("p (g c) -> p g c", c=65)[:, :, 64])
                    ts_(rsc[:, 8:16], rsc[:, 8:16], negl_t, None, OP.mult)
                    for qb in range(8):
                        b1, c1 = o_loc(0, qb)
                        b2, c2 = o_loc(1, qb)
                        ydst = yy[qb][:, 64 * h:64 * h + 64]
                        ts_(ydst, ot[b1][:, c1:c1 + 64],
                            rsc[:, qb:qb + 1], None, OP.mult)
                        stt(ydst, ot[b2][:, c2:c2 + 64],
                            rsc[:, 8 + qb:9 + qb], ydst, OP.mult, OP.add)

                # ---------- emission ----------
                # heads 0/1 q/k projections inline via the s-pool tiles
                # ([128,1024] psum, single wide epilogue) for a dense start.
                for h01, kind in ((0, "q"), (1, "q"), (0, "k"), (1, "k")):
                    w_t = wq_t if kind == "q" else wk_t
                    x_t = xq_t if kind == "q" else xk_t
                    dst = qp[h01] if kind == "q" else kp[h01]
                    bcol = bqk_t[:, h01:h01 + 1] if kind == "q" \
                        else bqk_t[:, 8 + h01:9 + h01]
                    wofs = (lambda r: 1536 * r + 192 * h01) if kind == "q" \
                        else (lambda r: 1024 * r + 128 * h01)
                    ps = ps_s.tile([128, S], f32, tag="s", name="s")
                    for n in range(2):
                        for r in range(4):
                            nc.tensor.matmul(
                                ps[:, 512 * n:512 * (n + 1)],
                                w_t[:, wofs(r):wofs(r) + 128],
                                x_t[:, S * r + 512 * n:S * r + 512 * (n + 1)],
                                start=(r == 0), stop=(r == 3))
                    ts_(dst, ps, bcol, None, OP.add)

                filler = []
                for j in range(NJ):
                    filler.extend(vg_units(j, "v"))
                for q in range(NQ):
                    filler.extend(vg_units(q, "g"))
                for h2 in range(2, 8):
                    filler.extend(qk_units(h2, "q"))
                    filler.extend(qk_units(h2, "k"))

                # software-pipelined attention loop with a 2-step o-lag:
                # everything PE does in step i depends only on exp(i-2), so
                # the PE never waits mid-step and the exp stream stays dense.
                steps = [(h, t, j) for h in range(8) for t in range(2)
                         for j in range(NJ)]
                otiles = {}
                pending = []

                def emit_o(ph, pt, pj, pex):
                    if pt == 0 and pj == 0:
                        otiles[ph] = [
                            ps_o.tile([128, 512], f32, tag=f"o{i}", name=f"o{i}")
                            for i in range(3)]
                    ot = otiles[ph]
                    for qb in range(NQ):
                        bi, c0 = o_loc(pt, qb)
                        # start only on the FIRST matmul into each bank this
                        # head: start_tensor_calc zeroes the whole bank, so
                        # later groups accumulate onto the zeroed regions.
                        first = (pj == 0) and (pt, qb) in ((0, 0), (0, 7), (1, 6))
                        nc.tensor.matmul(
                            ot[bi][:, c0:c0 + 65],
                            pex[:, 128 * qb:128 * (qb + 1)],
                            va[pj][:, ph, :],
                            start=first, stop=(pj == NJ - 1),
                            skip_group_check=True)
                    if pt == 1 and pj == NJ - 1:
                        combine(ph, ot)

                for idx, (h, t, j) in enumerate(steps):
                    # o-matmuls of step i-2 first: they are ready the moment
                    # exp(i-1) starts, so their leading stationary-switch
                    # stall hides under exp; scores of step i follow and
                    # complete well before exp(i-1) ends.
                    if len(pending) >= 2:
                        emit_o(*pending.pop(0))
                    s_t = ps_s.tile([128, S], f32, tag="s", name="s")
                    for n in range(2):
                        nc.tensor.matmul(
                            s_t[:, 512 * n:512 * (n + 1)],
                            kp[h][64 * t:64 * t + 64, 128 * j:128 * (j + 1)],
                            qp[h][64 * t:64 * t + 64, 512 * n:512 * (n + 1)],
                            start=True, stop=True)
                    ex = expp.tile([128, S], bf16, tag="ex", name="ex")
                    nc.scalar.activation(ex, s_t, AF.Exp, scale=INV)
                    pending.append((h, t, j, ex))
                    npop = 4 if idx < 8 else (2 if idx < 56 else 1)
                    for _ in range(npop):
                        if filler:
                            filler.pop(0)()
                while pending:
                    emit_o(*pending.pop(0))
                while filler:
                    filler.pop(0)()

            # ---------- tail: GroupNorm stats + affine + gate + output ----
            with tc.tile_pool(name="ps_tail", bufs=1, space="PSUM") as ps_t, \
                 tc.tile_pool(name="y2p", bufs=2) as y2p, \
                 tc.tile_pool(name="oqp", bufs=8) as oqp, \
                 tc.tile_pool(name="tsb", bufs=1) as tsb:

                sy = ps_t.tile([1, 512], f32, tag="sy", name="sy")
                sy2 = ps_t.tile([1, 512], f32, tag="sy2", name="sy2")
                mcol = ps_t.tile([8, 2], f32, tag="mcol", name="mcol")
                rxA = ps_t.tile([1, 512], f32, tag="rxA", name="rxA")
                rxB = ps_t.tile([1, 512], f32, tag="rxB", name="rxB")
                ab = ps_t.tile([128, 1024], f32, tag="ab", name="ab")

                for qb in range(NQ):
                    ybf = y2p.tile([128, 512], bf16, tag="ybf", name="ybf")
                    nc.scalar.copy(ybf, yy[qb])
                    nc.tensor.matmul(sy, ones_c[:], ybf,
                                     start=(qb == 0), stop=(qb == NQ - 1))
                    y2 = y2p.tile([128, 512], bf16, tag="y2", name="y2")
                    if qb % 2 == 0:
                        nc.vector.tensor_mul(y2, yy[qb], yy[qb])
                    else:
                        nc.gpsimd.tensor_mul(y2, yy[qb], yy[qb])
                    nc.tensor.matmul(sy2, ones_c[:], y2,
                                     start=(qb == 0), stop=(qb == NQ - 1))

                gsum = tsb.tile([1, 8], f32, tag="gsum", name="gsum")
                g2 = tsb.tile([1, 8], f32, tag="g2", name="g2")
                nc.vector.tensor_reduce(
                    gsum, sy.rearrange("o (h g e) -> o g h e", h=8, g=8),
                    axis=AX.XY, op=OP.add)
                nc.vector.tensor_reduce(
                    g2, sy2.rearrange("o (h g e) -> o g h e", h=8, g=8),
                    axis=AX.XY, op=OP.add)
                nc.tensor.matmul(mcol[:, 0:1], gsum, one1, start=True, stop=False,
                                 skip_group_check=True)
                nc.tensor.matmul(mcol[:, 1:2], g2, one1, start=False, stop=True,
                                 skip_group_check=True)

                e2t = tsb.tile([8, 1], f32, tag="e2t", name="e2t")
                nm = tsb.tile([8, 1], f32, tag="nm", name="nm")
                veps = tsb.tile([8, 1], f32, tag="veps", name="veps")
                sd = tsb.tile([8, 1], f32, tag="sd", name="sd")
                rsd = tsb.tile([8, 1], f32, tag="rsd", name="rsd")
                rr = tsb.tile([8, 1], f32, tag="rr", name="rr")
                ts_(mrst[:, 1:2], mcol[:, 0:1], 1.0 / float(S * H * 8), None, OP.mult)
                ts_(e2t, mcol[:, 1:2], 1.0 / float(S * H * 8), None, OP.mult)
                ts_(nm, mrst[:, 1:2], mrst[:, 1:2], -1.0, OP.mult, OP.mult)
                stt(veps, nm, EPS, e2t, OP.add, OP.add)
                nc.scalar.activation(sd, veps, AF.Sqrt)
                nc.vector.reciprocal(rsd, sd)
                nc.vector.tensor_mul(rr, rsd, rsd)
                nc.vector.tensor_mul(rr, rr, veps)
                ts_(rr, rr, -0.5, 1.5, OP.mult, OP.add)
                nc.vector.tensor_mul(mrst[:, 0:1], rsd, rr)

                nc.tensor.matmul(rxA, mrst[:, 0:1], g8_t[:, :], start=True, stop=True)
                nc.tensor.matmul(rxB, mrst[:, 1:2], g8_t[:, :], start=True, stop=True)
                arow = tsb.tile([1, 512], f32, tag="arow", name="arow")
                btmp = tsb.tile([1, 512], f32, tag="btmp", name="btmp")
                brow = tsb.tile([1, 512], f32, tag="brow", name="brow")
                nc.vector.tensor_mul(arow, ghl_r, rxA)
                nc.vector.tensor_mul(btmp, rxB, arow)
                nc.vector.tensor_sub(brow, bhl_r, btmp)
                nc.tensor.matmul(ab[:, 0:512], ones_rf[:], arow,
                                 start=True, stop=True)
                nc.tensor.matmul(ab[:, 512:1024], ones_rf[:], brow,
                                 start=True, stop=True)

                ab_sb = tsb.tile([128, 1024], f32, tag="ab_sb", name="ab_sb")
                nc.vector.tensor_copy(ab_sb[:, 0:512], ab[:, 0:512])
                nc.vector.tensor_copy(ab_sb[:, 512:1024], ab[:, 512:1024])
                for qb in (0, 5, 1, 6, 2, 7, 3, 4):
                    oq = oqp.tile([128, 512], f32, tag="oq", name="oq")
                    if qb < 5:
                        nc.vector.tensor_mul(oq, yy[qb], ab_sb[:, 0:512])
                        nc.vector.tensor_add(oq, oq, ab_sb[:, 512:1024])
                        stt(oq, th[qb], 1.0, oq, OP.add, OP.mult)
                    else:
                        nc.gpsimd.tensor_mul(oq, yy[qb], ab_sb[:, 0:512])
                        nc.gpsimd.tensor_add(oq, oq, ab_sb[:, 512:1024])
                        stt(oq, th[qb], 1.0, oq, OP.add, OP.mult)
                    nc.sync.dma_start(out=out_d[128 * qb:128 * (qb + 1), :], in_=oq)

    nc.finalize()
    return nc


_CACHE = {}


def _get_nc():
    if "nc" not in _CACHE:
        _CACHE["nc"] = build_nc()
    return _CACHE["nc"]


def _host_prep(arrs):
    """Pack weights/biases into device layouts (bf16 x^T chunks etc.)."""
    from ml_dtypes import bfloat16 as bf

    def rpack(w):  # [512, C] -> [128, 4*C] with [p, C*r + c] = w[128r + p, c]
        c = w.shape[1]
        return np.ascontiguousarray(
            w.reshape(4, 128, c).transpose(1, 0, 2).reshape(128, 4 * c)).astype(bf)

    wq, wk, wv = arrs["Wq"], arrs["Wk"], arrs["Wv"]
    wg = np.ascontiguousarray(wq.reshape(DM, 8, 192)[:, :, 128:].reshape(DM, 512))
    bq, bk, bv = arrs["bq"], arrs["bk"], arrs["bv"]
    lam = float(arrs["lam"][0])
    li = float(arrs["lambda_init"][0])
    hl = 0.5 * (1.0 - li)

    bqk = np.zeros((128, 16), np.float32)
    for h in range(8):
        bqk[:, h] = bq[192 * h:192 * h + 128]
        bqk[:, 8 + h] = bk[128 * h:128 * h + 128]
    gb = bq.reshape(8, 192)[:, 128:].reshape(512)
    rows = np.concatenate([
        np.tile(arrs["gamma"], 8) * hl,
        np.tile(arrs["beta"], 8) * hl,
        gb, bv]).astype(np.float32).reshape(1, 2048)
    g8 = np.zeros((8, 512), np.float32)
    cols = np.arange(512)
    g8[(cols % 64) // 8, cols] = 1.0

    rowsb = np.concatenate([gb, bv]).astype(bf).reshape(1, 1024)
    shared = {
        "wq": rpack(wq), "wk": rpack(wk), "wv": rpack(wv), "wg": rpack(wg),
        "bqk": np.ascontiguousarray(bqk), "rows": rows, "rowsb": rowsb,
        "g8": g8, "negl": np.array([-lam], np.float32),
    }
    in_maps = []
    for i in range(B):
        m = dict(shared)
        for nm, key in (("xq", "query"), ("xk", "key"), ("xv", "values")):
            m[nm] = rpack(np.ascontiguousarray(arrs[key][i].T))
        in_maps.append(m)
    return in_maps


def run(inputs, trace=False, tmpdir=None):
    from concourse.bass_utils import run_bass_kernel_spmd
    nc = _get_nc()
    arrs = {k: np.asarray(v, dtype=np.float32) for k, v in inputs.items()}
    in_maps = _host_prep(arrs)
    res = run_bass_kernel_spmd(nc, in_maps, core_ids=list(range(B)),
                               trace=trace, tmpdir=tmpdir)
    out = np.stack([res.results[i]["out"] for i in range(B)], axis=0)
    return out.astype(np.float32), res


def kernel(**inputs):
    out, _ = run(inputs)
    return out
